# revision 34
# baseline (speedup 1.0000x reference)
"""KNNGraph (k=16) Bass kernel for 8 NeuronCores.

Input: x (4, 8192, 64) fp32. Output: (src, dst) int32 edge arrays of the
16-NN graph per batch (self included), matching jax.lax.top_k(-d2) order.

Sharding: core c handles batch c//2, query rows (c%2)*4096 ... +4096,
against all 8192 keys of that batch (query-row sharding, keys replicated).

Device pipeline (per core), for each of 32 groups of 128 query rows,
with a 2-level max tournament so the DVE only scans 1/4 of the matrix
with MAX8/FIND_INDEX8 (which run at 1 elem/cycle, vs TENSOR_TENSOR max
which streams 2 elems/cycle):
  PE  : w = q.k - |k|^2/2 per 1024-key window (fp16 hi/lo split inputs,
        K=128 contraction, 2x N=512 matmuls into one PSUM tile)
  ACT : copy w PSUM -> SBUF
  DVE : m1_w = max(w[:, 0:512], w[:, 512:1024])          (512 pairs)
        m2_w = max(m1_w[:, 0:256], m1_w[:, 256:512])     (256 4-key groups)
        per window: MAX8 + FIND_INDEX8 on the 256-wide m2_w
        -> 64 (group-value, group-index) candidates per row
        group p of window w covers keys w*1024 + {p, p+256, p+512, p+768}
Host: expand top-24 groups to 96 keys, exact rescore (fp32 dots, fp64
      combine), order by (d2, idx); conservative suspect checks ->
      exact fallback recompute for flagged rows.
"""

import numpy as np

N, M, D = 4, 8192, 64
K = 16
NCORES = 8
QROWS = M // 2           # query rows per core
NGROUPS = QROWS // 128   # 32
NWIN = 8                 # windows of 1024 keys
WIN = M // NWIN          # 1024
PWIN = WIN // 2          # 512 pairs per window (m1 plane)
GWIN = WIN // 4          # 256 4-key groups per window (m2 plane)
HWIN = WIN // 8          # 128 8-key groups per window (m3 plane)
SWIN = WIN // 16         # 64 16-key groups per window (m4 plane)
KDIM = 128               # contraction rows
NCAND = NWIN * 8         # 64 group candidates per row
# A group holding a top-16 key always ranks <=16 among groups by group-max
# (at most 15 other keys beat any of its members), so 16 + margin slack.
RESCORE = 16             # exact-rescored group candidates per row (x16 keys)

# gpsimd reads the pairwise max straight from PSUM; set False to route
# through an ACT copy to SBUF instead.
GPSIMD_FROM_PSUM = False

_COMPILED = {}
LAST_RUN = {}


def _build_nc():
    import concourse.bacc as bacc
    import concourse.mybir as mybir
    import concourse.tile as tile

    nc = bacc.Bacc(None)
    f32 = mybir.dt.float32
    f16 = mybir.dt.float16
    u32 = mybir.dt.uint32
    Act = mybir.ActivationFunctionType

    q_d = nc.declare_dram_parameter("q", [KDIM, QROWS], f16, isOutput=False)
    kv_d = nc.declare_dram_parameter("kv", [KDIM, M], f16, isOutput=False)
    cv_d = nc.declare_dram_parameter("cv", [NGROUPS, 128, NCAND], f16, isOutput=True)
    ci_d = nc.declare_dram_parameter("ci", [NGROUPS, 128, NCAND], u32, isOutput=True)

    with tile.TileContext(nc) as tc:
        with (
            tc.tile_pool(name="singles", bufs=1) as singles,
            tc.tile_pool(name="psum", bufs=2, space="PSUM") as psum,
            tc.tile_pool(name="wcopy", bufs=4) as wcopy,
            tc.tile_pool(name="mplane", bufs=3) as mpool,
            tc.tile_pool(name="cands", bufs=2) as cands,
        ):
            q_sb = singles.tile([KDIM, QROWS], f16)
            kv_sb = singles.tile([KDIM, M], f16)
            nc.gpsimd.dma_start(out=q_sb[:], in_=q_d[:])
            nc.gpsimd.dma_start(out=kv_sb[:], in_=kv_d[:])

            for g in range(NGROUPS):
                cv = cands.tile([128, NCAND], f16, tag="cv")
                ci = cands.tile([128, NCAND], u32, tag="ci")
                m = mpool.tile([128, NWIN * PWIN], f16, tag="m")    # pair maxes
                m2 = mpool.tile([128, NWIN * GWIN], f16, tag="m2")  # 4-key maxes
                wt = wcopy.tile([128, M], f16, tag="wt")
                lhsT = q_sb[:, g * 128:(g + 1) * 128]
                for t in range(M // 2048):      # 2 windows per PSUM tile
                    pt = psum.tile([128, 2048], f32, tag="pt")
                    for hh in range(2048 // 512):
                        j0 = t * 2048 + hh * 512
                        nc.tensor.matmul(
                            pt[:, hh * 512:(hh + 1) * 512], lhsT,
                            kv_sb[:, j0:j0 + 512], start=True, stop=True,
                        )
                    nc.scalar.activation(
                        out=wt[:, t * 2048:(t + 1) * 2048], in_=pt[:], func=Act.Copy
                    )
                # level-1 combine, all 8 windows in one 3D-AP op:
                # m1[w][p] = max(w[w][p], w[w][p+512])
                wv = wt[:].rearrange("p (w u) -> p w u", w=NWIN, u=WIN)
                mv = m[:].rearrange("p (w u) -> p w u", w=NWIN, u=PWIN)
                nc.vector.tensor_max(mv, wv[:, :, 0:PWIN], wv[:, :, PWIN:WIN])
                # level-2 combine: m2[w][p] = max(m1[w][p], m1[w][p+256])
                m2v = m2[:].rearrange("p (w u) -> p w u", w=NWIN, u=GWIN)
                nc.vector.tensor_max(m2v, mv[:, :, 0:GWIN], mv[:, :, GWIN:PWIN])
                # level-3 combine: m3[w][p] = max(m2[w][p], m2[w][p+128])
                m3 = mpool.tile([128, NWIN * HWIN], f16, tag="m3")
                m3v = m3[:].rearrange("p (w u) -> p w u", w=NWIN, u=HWIN)
                nc.vector.tensor_max(m3v, m2v[:, :, 0:HWIN], m2v[:, :, HWIN:GWIN])
                # level-4 combine: m4[w][p] = max(m3[w][p], m3[w][p+64])
                m4 = mpool.tile([128, NWIN * SWIN], f16, tag="m4")
                m4v = m4[:].rearrange("p (w u) -> p w u", w=NWIN, u=SWIN)
                nc.vector.tensor_max(m4v, m3v[:, :, 0:SWIN], m3v[:, :, SWIN:HWIN])
                for w in range(NWIN):
                    m4w = m4[:, w * SWIN:(w + 1) * SWIN]
                    nc.vector.max(out=cv[:, w * 8:(w + 1) * 8], in_=m4w)
                for w in range(NWIN):
                    m4w = m4[:, w * SWIN:(w + 1) * SWIN]
                    nc.vector.max_index(
                        out=ci[:, w * 8:(w + 1) * 8],
                        in_max=cv[:, w * 8:(w + 1) * 8],
                        in_values=m4w,
                    )
                nc.sync.dma_start(out=cv_d[g], in_=cv[:])
                nc.sync.dma_start(out=ci_d[g], in_=ci[:])
    if not nc.is_finalized():
        nc.finalize()
    return nc


def _split16(a):
    """fp16 hi/lo split of float64 array -> (hi, lo) fp16."""
    hi = a.astype(np.float16)
    lo = (a - hi.astype(np.float64)).astype(np.float16)
    return hi, lo


def _prep_inputs(x):
    """Per-core input dicts. x: (N, M, D) fp32."""
    x64 = x.astype(np.float64)
    qhi, qlo = _split16(x64)                     # (N, M, D)
    nrm = -0.5 * (x64 * x64).sum(-1)             # (N, M)
    nh, nl = _split16(nrm)

    in_maps = []
    for c in range(NCORES):
        b, h = c // 2, c % 2
        sl = slice(h * QROWS, (h + 1) * QROWS)
        q = np.zeros((KDIM, QROWS), np.float16)
        q[:D] = qhi[b, sl, :].T
        q[D:D + 61] = qlo[b, sl, :61].T
        q[125] = nrm[b, sl]          # -|q|^2/2: shifts PSUM to -d2/2
        q[126] = 1.0
        q[127] = 1.0
        kv = np.zeros((KDIM, M), np.float16)
        kv[:D] = qhi[b].T
        kv[D:D + 61] = qhi[b, :, :61].T
        kv[125] = 1.0
        kv[126] = nh[b]
        kv[127] = nl[b]
        in_maps.append({"q": q, "kv": kv})
    return in_maps


def kernel(x, k):
    x = np.asarray(x, dtype=np.float32)
    k = int(k)
    assert x.shape == (N, M, D) and k == K

    from concourse.bass_utils import run_bass_kernel_spmd

    if "nc" not in _COMPILED:
        _COMPILED["nc"] = _build_nc()
    nc = _COMPILED["nc"]

    in_maps = _prep_inputs(x)
    _r = run_bass_kernel_spmd(nc, in_maps, list(range(NCORES)))
    LAST_RUN["results"] = _r
    res = _r.results

    cv = np.empty((N, M, NCAND), np.float32)   # pair-max values
    ci = np.empty((N, M, NCAND), np.int64)     # pair idx within window (0..511)
    for c in range(NCORES):
        b, h = c // 2, c % 2
        sl = slice(h * QROWS, (h + 1) * QROWS)
        cv[b, sl] = res[c]["cv"].reshape(QROWS, NCAND)
        ci[b, sl] = res[c]["ci"].reshape(QROWS, NCAND)

    x64 = x.astype(np.float64)

    # ---- host merge: top-RESCORE pairs by value, expand to keys, rescore ----
    order = np.argsort(-cv, axis=-1, kind="stable")             # (N, M, 64)
    top = order[..., :RESCORE]
    pwin = top >> 3                                             # window id (0..7)
    ploc = np.take_along_axis(ci, top, axis=-1)                 # (N, M, 24)
    keyA = pwin * WIN + ploc                                    # first member
    kidx = np.empty((N, M, 16 * RESCORE), np.int64)             # 256 keys
    for j in range(16):
        kidx[..., j::16] = keyA + j * SWIN

    x2_64 = (x64 * x64).sum(-1)                                 # (N, M) exact-ish
    idx16 = np.empty((N, M, K), np.int64)
    d2_16 = np.empty((N, M, K), np.float64)
    for b in range(N):
        keys = x[b][kidx[b]]                                    # (M, 144, 64) fp32
        dots = np.matmul(keys, x[b][:, :, None])[..., 0]        # batched matvec
        d2 = x2_64[b][:, None] + x2_64[b][kidx[b]] - 2.0 * dots.astype(np.float64)
        perm = np.lexsort((kidx[b], d2), axis=-1)[:, :K]
        idx16[b] = np.take_along_axis(kidx[b], perm, axis=-1)
        d2_16[b] = np.take_along_axis(d2, perm, axis=-1)

    # ---- suspect detection --------------------------------------------
    # device values are w' = -d2/2 (+ per-row fp16 shift error <=0.03,
    # + fp16 quantization <=0.03 in the interesting range)
    w16 = -0.5 * d2_16[..., K - 1:K]                            # w' of 16th
    MARGIN = 0.15
    win8 = cv[..., 7::8].astype(np.float64)
    suspect = (win8 >= w16 - MARGIN).any(-1)
    v_sorted = np.take_along_axis(cv, order, axis=-1).astype(np.float64)
    suspect |= (v_sorted[..., RESCORE] >= w16[..., 0] - MARGIN)
    sv = np.sort(idx16, axis=-1)
    suspect |= (sv[..., 1:] == sv[..., :-1]).any(-1)
    # fp16-equal values inside one window's top-8 collapse FIND_INDEX8
    # positions (a candidate group is lost) -- only relevant if the tied
    # value could still be a winner
    cvw = cv.reshape(N, M, NWIN, 8)
    dup = (cvw[..., 1:] == cvw[..., :-1]) & (
        cvw[..., 1:].astype(np.float64) >= (w16 - MARGIN)[..., None]
    )
    suspect |= dup.any(-1).any(-1)

    nbad = int(suspect.sum())
    if nbad:
        q2_32 = (x * x).sum(-1)                                 # fp32 norms
        for b in range(N):
            rows = np.nonzero(suspect[b])[0]
            if rows.size == 0:
                continue
            for c0 in range(0, rows.size, 2048):
                rr = rows[c0:c0 + 2048]
                d2r = (
                    q2_32[b][rr][:, None] + q2_32[b][None, :]
                    - 2.0 * (x[b][rr] @ x[b].T)
                )                                               # (r, M) fp32
                part = np.argpartition(d2r, K + 8, axis=-1)[:, : K + 8]
                keys = x64[b][part]                             # (r, 24, 64)
                dif = keys - x64[b][rr][:, None, :]
                pd = np.einsum("rcd,rcd->rc", dif, dif)         # exact fp64
                pperm = np.lexsort((part, pd), axis=-1)[:, :K]
                idx16[b, rr] = np.take_along_axis(part, pperm, axis=-1)

    offset = (np.arange(N, dtype=np.int64) * M)[:, None, None]
    src = (idx16 + offset).reshape(-1).astype(np.int32)
    dst = np.repeat(np.arange(N * M, dtype=np.int32), K)
    return src, dst


if __name__ == "__main__":
    rng = np.random.default_rng(0)
    xt = rng.standard_normal((N, M, D), dtype=np.float32)
    s, d = kernel(xt, 16)
    print(s[:32], d[:32])
